# revision 3
# baseline (speedup 1.0000x reference)
"""Trainium2 Bass kernel for nn_ActorNetwork (conv3d + Tucker/HOOI + FC + softmax).

Structure (8 NeuronCores, D-dim sharded 64 -> 8 per core):
  NEFF-A (1 launch): conv3d(k=(1,3,1)) + bias + relu via TensorE matmul
      (K=48 block-diag weights -> 128 output partitions), writes y in two
      DRAM layouts: y_hb [h=128, (c,d,w)] and y_cdb [(dq,c)=128, q, (h,w)].
  NEFF-B (N_SWEEPS+1 launches): one Jacobi-HOOI sweep's device work:
      TA2 = y x2 U2 x3 U3   (per-core local-d block; contract h then w)
      TC  = y x0 U0 x1 U1l  (partial sum over local d; Kronecker V trick)
  Host between launches: assemble projections, small Grams, LAPACK eigh
      (eigh must be host LAPACK: eigenvector signs must match the CPU
      reference; the neuron backend cannot lower eigh at all).
  Final: core tensor from last launch's TA2, FC + softmax on host.

y stays device-resident between launches as jax arrays (custom PJRT runner).
"""

import os
import numpy as np

N_CORES = 8
RANKS = (8, 6, 6, 6)
N_SWEEPS = int(os.environ.get("KERNEL_N_SWEEPS", "12"))
C, D, H, W = 32, 64, 128, 64  # y dims; x is (4, 64, 130, 64)
DL = D // N_CORES  # 8 local d per core

_cache = {}
stash = {}  # test.py introspection


# ----------------------------------------------------------------- builders
def _build_neff_a():
    import concourse.tile as tile
    from concourse import bacc, mybir

    nc = bacc.Bacc("TRN2", target_bir_lowering=False, debug=False,
                   num_devices=N_CORES)
    dt = mybir.dt.float32
    x_im = nc.dram_tensor("x_im", [96, 8192], dt, kind="ExternalInput").ap()
    w48 = nc.dram_tensor("w48", [48, 128], dt, kind="ExternalInput").ap()
    b128 = nc.dram_tensor("b128", [128, 1], dt, kind="ExternalInput").ap()
    y_hb = nc.dram_tensor("y_hb", [2097152], dt, kind="ExternalOutput").ap()
    y_cdb = nc.dram_tensor("y_cdb", [128, 16384], dt, kind="ExternalOutput").ap()

    # y_hb flat = h*16384 + c*512 + d*64 + w ; d = 2*dq + q ; p = c*4+dq
    # => flat = h*16384 + (c*4+dq)*128 + q*64 + w
    y_hb_v = y_hb.rearrange("(h cdq q w) -> q cdq h w",
                            h=128, cdq=128, q=2, w=64)

    with tile.TileContext(nc) as tcx:
        with (
            tcx.tile_pool(name="const", bufs=1) as constp,
            tcx.tile_pool(name="imc", bufs=2) as imcp,
            tcx.tile_pool(name="ysb", bufs=2) as ysbp,
            tcx.tile_pool(name="ps", bufs=8, space="PSUM") as psp,
        ):
            w_sb = constp.tile([48, 128], dt)
            nc.sync.dma_start(w_sb[:], w48[:])
            b_sb = constp.tile([128, 1], dt)
            nc.sync.dma_start(b_sb[:], b128[:])

            for q in range(2):
                imc = imcp.tile([48, 8192], dt, tag="imc")
                nc.sync.dma_start(imc[:], x_im[q * 48:(q + 1) * 48, :])
                y_sb = ysbp.tile([128, 8192], dt, tag="ysb")
                for n in range(16):
                    ps = psp.tile([128, 512], dt, tag="ps")
                    nc.tensor.matmul(ps[:], w_sb[:], imc[:, n * 512:(n + 1) * 512],
                                     start=True, stop=True)
                    nc.scalar.activation(y_sb[:, n * 512:(n + 1) * 512], ps[:],
                                         mybir.ActivationFunctionType.Relu,
                                         bias=b_sb[:])
                nc.sync.dma_start(y_cdb[:, q * 8192:(q + 1) * 8192], y_sb[:])
                nc.sync.dma_start(y_hb_v[q], y_sb[:])
    nc.compile()
    return nc


def _build_neff_b():
    import concourse.tile as tile
    from concourse import bacc, mybir

    nc = bacc.Bacc("TRN2", target_bir_lowering=False, debug=False,
                   num_devices=N_CORES)
    dt = mybir.dt.float32
    y_hb = nc.dram_tensor("y_hb", [2097152], dt, kind="ExternalInput").ap()
    y_cdb = nc.dram_tensor("y_cdb", [128, 16384], dt, kind="ExternalInput").ap()
    u2 = nc.dram_tensor("u2", [128, 6], dt, kind="ExternalInput").ap()
    u3d = nc.dram_tensor("u3d", [128, 12], dt, kind="ExternalInput").ap()
    v2 = nc.dram_tensor("v2", [128, 96], dt, kind="ExternalInput").ap()
    ta2 = nc.dram_tensor("ta2", [12, 768], dt, kind="ExternalOutput").ap()
    tc_out = nc.dram_tensor("tc", [48, 8192], dt, kind="ExternalOutput").ap()

    y_hb2 = y_hb.rearrange("(p f) -> p f", p=128)

    with tile.TileContext(nc) as tcx:
        with (
            tcx.tile_pool(name="const", bufs=1) as constp,
            tcx.tile_pool(name="yh", bufs=3) as yhp,
            tcx.tile_pool(name="ycd", bufs=1) as ycdp,
            tcx.tile_pool(name="ta1", bufs=1) as ta1p,
            tcx.tile_pool(name="out", bufs=1) as outp,
            tcx.tile_pool(name="ps1", bufs=2, space="PSUM") as ps1p,
            tcx.tile_pool(name="psa", bufs=2, space="PSUM") as psap,
            tcx.tile_pool(name="ps2", bufs=4, space="PSUM") as ps2p,
        ):
            u2_sb = constp.tile([128, 6], dt)
            nc.sync.dma_start(u2_sb[:], u2[:])
            u3d_sb = constp.tile([128, 12], dt)
            nc.sync.dma_start(u3d_sb[:], u3d[:])
            v2_sb = constp.tile([128, 96], dt)
            nc.sync.dma_start(v2_sb[:], v2[:])

            # --- TC path: Kronecker contraction over (c, d_local)
            ycd = []
            for b in range(2):
                t = ycdp.tile([128, 8192], dt, tag=f"ycd{b}")
                nc.sync.dma_start(t[:], y_cdb[:, b * 8192:(b + 1) * 8192])
                ycd.append(t)
            tc_sb = outp.tile([48, 8192], dt, tag="tcsb")
            for k in range(16):
                ps = ps2p.tile([48, 512], dt, tag="ps2")
                nc.tensor.matmul(ps[:], v2_sb[:, 0:48],
                                 ycd[0][:, k * 512:(k + 1) * 512],
                                 start=True, stop=False)
                nc.tensor.matmul(ps[:], v2_sb[:, 48:96],
                                 ycd[1][:, k * 512:(k + 1) * 512],
                                 start=False, stop=True)
                nc.vector.tensor_copy(tc_sb[:, k * 512:(k + 1) * 512], ps[:])
            nc.sync.dma_start(tc_out[:], tc_sb[:])

            # --- TA path: contract h (partitions), then w (doubled U3)
            ta1_sb = ta1p.tile([128, 768], dt)
            for g in range(4):
                yh = yhp.tile([128, 4096], dt, tag="yh")
                nc.sync.dma_start(yh[:], y_hb2[:, g * 4096:(g + 1) * 4096])
                ps = ps1p.tile([128, 192], dt, tag="ps1")
                for jj in range(32):
                    nc.tensor.matmul(ps[:, jj * 6:(jj + 1) * 6],
                                     yh[:, jj * 128:(jj + 1) * 128], u2_sb[:],
                                     start=True, stop=True)
                nc.vector.tensor_copy(ta1_sb[:, g * 192:(g + 1) * 192], ps[:])
            ta2_sb = outp.tile([12, 768], dt, tag="ta2sb")
            for hf in range(2):
                psa = psap.tile([12, 384], dt, tag="psa")
                nc.tensor.matmul(psa[:], u3d_sb[:],
                                 ta1_sb[:, hf * 384:(hf + 1) * 384],
                                 start=True, stop=True)
                nc.vector.tensor_copy(ta2_sb[:, hf * 384:(hf + 1) * 384], psa[:])
            nc.sync.dma_start(ta2[:], ta2_sb[:])
    nc.compile()
    return nc


# ----------------------------------------------------------------- runner
def _make_runner(nc):
    """jit once; inputs may be jax arrays (device-resident) or np arrays.
    Returns dict name -> global jax array ((N_CORES*dim0, ...))."""
    import jax
    import jax.numpy as jnp
    from jax.sharding import Mesh, PartitionSpec, NamedSharding
    from concourse import bass2jax, mybir

    bass2jax.install_neuronx_cc_hook()
    partition_name = (nc.partition_id_tensor.name
                      if nc.partition_id_tensor else None)
    in_names, out_names, out_avals = [], [], []
    for alloc in nc.m.functions[0].allocations:
        if not isinstance(alloc, mybir.MemoryLocationSet):
            continue
        name = alloc.memorylocations[0].name
        if alloc.kind == "ExternalInput":
            if name != partition_name:
                in_names.append(name)
        elif alloc.kind == "ExternalOutput":
            shape = tuple(alloc.tensor_shape)
            dtype = mybir.dt.np(alloc.dtype)
            out_names.append(name)
            out_avals.append(jax.core.ShapedArray(shape, dtype))
    n_params = len(in_names)
    all_names = tuple(in_names + out_names
                      + ([partition_name] if partition_name else []))
    donate = tuple(range(n_params, n_params + len(out_names)))

    def _body(*args):
        operands = list(args)
        if partition_name:
            operands.append(bass2jax.partition_id_tensor())
        outs = bass2jax._bass_exec_p.bind(
            *operands, out_avals=tuple(out_avals), in_names=all_names,
            out_names=tuple(out_names), lowering_input_output_aliases=(),
            sim_require_finite=True, sim_require_nnan=True, nc=nc)
        return tuple(outs)

    devices = jax.devices()[:N_CORES]
    mesh = Mesh(np.asarray(devices), ("core",))
    specs_in = (PartitionSpec("core"),) * (n_params + len(out_names))
    specs_out = (PartitionSpec("core"),) * len(out_names)
    try:
        from jax.experimental.shard_map import shard_map
    except ImportError:
        from jax import shard_map
    fn = jax.jit(shard_map(_body, mesh=mesh, in_specs=specs_in,
                           out_specs=specs_out, check_rep=False),
                 donate_argnums=donate, keep_unused=True)
    shd = NamedSharding(mesh, PartitionSpec("core"))
    zfn = jax.jit(
        lambda: tuple(jnp.zeros((N_CORES * a.shape[0],) + a.shape[1:], a.dtype)
                      for a in out_avals),
        out_shardings=(shd,) * len(out_avals))

    def run(inputs):
        args = [inputs[n] for n in in_names]
        outs = fn(*args, *zfn())
        return dict(zip(out_names, outs))

    run.in_names = in_names
    run.out_names = out_names
    return run


# ----------------------------------------------------------------- host math
def _mode_dot_t(t, u, mode):
    return np.moveaxis(np.tensordot(t, u, axes=(mode, 0)), -1, mode).astype(np.float32)


def _top_evecs(G, rank):
    _, v = np.linalg.eigh(G)
    return np.ascontiguousarray(v[:, ::-1][:, :rank]).astype(np.float32)


def _unfold(t, mode):
    return np.moveaxis(t, mode, 0).reshape(t.shape[mode], -1)


def _factor_inputs(U0, U1, U2, U3):
    """Per-launch small device inputs built from current factors (global)."""
    u2g = np.tile(U2, (N_CORES, 1))                       # (8*128, 6)
    u3d = np.zeros((128, 12), np.float32)
    u3d[:64, 0:6] = U3
    u3d[64:, 6:12] = U3
    u3dg = np.tile(u3d, (N_CORES, 1))
    v2g = np.zeros((N_CORES, 128, 96), np.float32)
    for ci in range(N_CORES):
        u1l = U1[ci * DL:(ci + 1) * DL]                   # (8, 6)
        for b in range(2):
            # p = c*4 + dq ; col = b*48 + r0*6 + r1 ; d_local = 2*dq + b
            kr = np.einsum('cr,ds->cdrs', U0, u1l[b::2])
            v2g[ci, :, b * 48:(b + 1) * 48] = kr.reshape(128, 48)
    return u2g, u3dg, v2g.reshape(N_CORES * 128, 96)


def _decode(outs):
    """Global device outputs -> TA2f (32,64,6,6), TCf (8,6,128,64)."""
    ta2 = np.asarray(outs["ta2"]).reshape(N_CORES, 12, 768)
    tcg = np.asarray(outs["tc"]).reshape(N_CORES, 48, 8192)
    t = ta2.reshape(N_CORES, 2, 6, 128, 6)                # [ci, b, r3, j, r2]
    t = t.transpose(0, 3, 1, 4, 2)                        # [ci, j, b, r2, r3]
    t = t.reshape(N_CORES, 32, 8, 6, 6)                   # cd=2j+b=(c, dl)
    TA2f = t.transpose(1, 0, 2, 3, 4).reshape(32, 64, 6, 6)
    TCf = tcg.sum(axis=0, dtype=np.float32).reshape(8, 6, 128, 64)
    return np.ascontiguousarray(TA2f), TCf


# ----------------------------------------------------------------- kernel
def kernel(x, conv_w, conv_b, fc_w, fc_b):
    x = np.asarray(x, np.float32)
    conv_w = np.asarray(conv_w, np.float32)
    conv_b = np.asarray(conv_b, np.float32)
    fc_w = np.asarray(fc_w, np.float32)
    fc_b = np.asarray(fc_b, np.float32)

    if "runA" not in _cache:
        _cache["ncA"] = _build_neff_a()
        _cache["ncB"] = _build_neff_b()
        _cache["runA"] = _make_runner(_cache["ncA"])
        _cache["runB"] = _make_runner(_cache["ncB"])
    runA, runB = _cache["runA"], _cache["runB"]

    # host-side input staging ------------------------------------------------
    # x_im[ci, q*48 + dq*12 + kh*4 + i, h*64+w] = x[i, 8ci+2dq+q, h+kh, w]
    x_im = np.empty((N_CORES, 96, 8192), np.float32)
    for ci in range(N_CORES):
        for q in range(2):
            for dq in range(4):
                d = 8 * ci + 2 * dq + q
                for kh in range(3):
                    r0 = q * 48 + dq * 12 + kh * 4
                    x_im[ci, r0:r0 + 4, :] = \
                        x[:, d, kh:kh + 128, :].reshape(4, 8192)
    w48 = np.zeros((48, 128), np.float32)
    for dq in range(4):
        for kh in range(3):
            for i in range(4):
                w48[dq * 12 + kh * 4 + i, dq::4] = conv_w[:, i, 0, kh, 0]
    b128 = np.repeat(conv_b, 4).reshape(128, 1).astype(np.float32)

    inA = {
        "x_im": x_im.reshape(N_CORES * 96, 8192),
        "w48": np.tile(w48, (N_CORES, 1)),
        "b128": np.tile(b128, (N_CORES, 1)),
    }
    stash["inA"] = inA
    outsA = runA(inA)
    y_hb_g, y_cdb_g = outsA["y_hb"], outsA["y_cdb"]
    stash["y_hb"] = y_hb_g
    stash["y_cdb"] = y_cdb_g

    # Jacobi-HOOI, identity init, host eigh ---------------------------------
    U0 = np.eye(C, dtype=np.float32)[:, :RANKS[0]]
    U1 = np.eye(D, dtype=np.float32)[:, :RANKS[1]]
    U2 = np.eye(H, dtype=np.float32)[:, :RANKS[2]]
    U3 = np.eye(W, dtype=np.float32)[:, :RANKS[3]]

    for sweep in range(N_SWEEPS + 1):
        u2g, u3dg, v2g = _factor_inputs(U0, U1, U2, U3)
        inB = {"y_hb": y_hb_g, "y_cdb": y_cdb_g,
               "u2": u2g, "u3d": u3dg, "v2": v2g}
        if sweep == 0:
            stash["inB_small"] = {"u2": u2g, "u3d": u3dg, "v2": v2g}
        outs = runB(inB)
        TA2f, TCf = _decode(outs)
        P0 = _mode_dot_t(TA2f, U1, 1)                     # (32, 6, 6, 6)
        if sweep == N_SWEEPS:
            core = np.tensordot(U0, P0, axes=(0, 0))      # (8, 6, 6, 6)
            break
        P1 = np.moveaxis(np.tensordot(TA2f, U0, axes=(0, 0)), -1, 0)
        P2 = _mode_dot_t(TCf, U3, 3)                      # (8, 6, 128, 6)
        P3 = _mode_dot_t(TCf, U2, 2)                      # (8, 6, 6, 64)
        newU = []
        for mode, P in ((0, P0), (1, P1), (2, P2), (3, P3)):
            M = _unfold(P, mode)
            G = (M @ M.T).astype(np.float32)
            newU.append(_top_evecs(G, RANKS[mode]))
        U0, U1, U2, U3 = newU

    logits = core.reshape(-1) @ fc_w.T + fc_b
    e = np.exp(logits - logits.max())
    return (e / e.sum()).astype(np.float32)


# revision 5
# speedup vs baseline: 1.5813x; 1.5813x over previous
"""Trainium2 Bass kernel for nn_ActorNetwork (conv3d + Tucker/HOOI + FC + softmax).

Structure (8 NeuronCores, D-dim sharded 64 -> 8 per core):
  NEFF-A (1 launch): conv3d(k=(1,3,1)) + bias + relu via TensorE matmul
      (K=48 block-diag weights -> 128 output partitions, fp32), cast to
      bf16, write y in two DRAM layouts:
      y_hb [h=128, (c,d,w)] and y_cdb [(c,dq)=128, q, (h,w)], d = 2*dq+q.
  NEFF-B (N_SWEEPS+1 launches): one Jacobi-HOOI sweep's device work
      (bf16 matmuls, fp32 PSUM):
      TA1 = y x2 U2   [6, (c,d,w)] per-core local-d block (contract h)
      TC  = y x0 U0 x1 U1l  [48, (h,w)] partial sum over local d (Kronecker)
  Host between launches: w-contraction of TA1, projections, small Grams,
      LAPACK eigh (host LAPACK required: eigenvector signs must match the
      CPU reference; the neuron backend cannot lower eigh at all).
  Final: core tensor from last launch, FC + softmax on host.

y stays device-resident between launches as jax arrays (custom PJRT runner).
"""

import os
import numpy as np
import ml_dtypes

BF16 = ml_dtypes.bfloat16
N_CORES = 8
RANKS = (8, 6, 6, 6)
N_SWEEPS = int(os.environ.get("KERNEL_N_SWEEPS", "12"))
C, D, H, W = 32, 64, 128, 64  # y dims; x is (4, 64, 130, 64)
DL = D // N_CORES

_cache = {}
stash = {}


# ----------------------------------------------------------------- builders
def _build_neff_a():
    import concourse.tile as tile
    from concourse import bacc, mybir

    nc = bacc.Bacc("TRN2", target_bir_lowering=False, debug=False,
                   num_devices=N_CORES)
    f32, bf = mybir.dt.float32, mybir.dt.bfloat16
    x_im = nc.dram_tensor("x_im", [96, 8192], f32, kind="ExternalInput").ap()
    w48 = nc.dram_tensor("w48", [48, 128], f32, kind="ExternalInput").ap()
    b128 = nc.dram_tensor("b128", [128, 1], f32, kind="ExternalInput").ap()
    y_hb = nc.dram_tensor("y_hb", [2097152], bf, kind="ExternalOutput").ap()
    y_cdb = nc.dram_tensor("y_cdb", [128, 16384], bf, kind="ExternalOutput").ap()

    # y_hb flat = h*16384 + c*512 + d*64 + w ; d = 2*dq + q ; p = c*4+dq
    y_hb_v = y_hb.rearrange("(h cdq q w) -> q cdq h w",
                            h=128, cdq=128, q=2, w=64)

    with tile.TileContext(nc) as tcx:
        with (
            tcx.tile_pool(name="const", bufs=1) as constp,
            tcx.tile_pool(name="imc", bufs=2) as imcp,
            tcx.tile_pool(name="ysb", bufs=2) as ysbp,
            tcx.tile_pool(name="ps", bufs=8, space="PSUM") as psp,
        ):
            w_sb = constp.tile([48, 128], f32)
            nc.sync.dma_start(w_sb[:], w48[:])
            b_sb = constp.tile([128, 1], f32)
            nc.sync.dma_start(b_sb[:], b128[:])

            for q in range(2):
                imc = imcp.tile([48, 8192], f32, tag="imc")
                nc.sync.dma_start(imc[:], x_im[q * 48:(q + 1) * 48, :])
                y_sb = ysbp.tile([128, 8192], bf, tag="ysb")
                for n in range(16):
                    ps = psp.tile([128, 512], f32, tag="ps")
                    nc.tensor.matmul(ps[:], w_sb[:], imc[:, n * 512:(n + 1) * 512],
                                     start=True, stop=True)
                    nc.scalar.activation(y_sb[:, n * 512:(n + 1) * 512], ps[:],
                                         mybir.ActivationFunctionType.Relu,
                                         bias=b_sb[:])
                nc.sync.dma_start(y_cdb[:, q * 8192:(q + 1) * 8192], y_sb[:])
                nc.sync.dma_start(y_hb_v[q], y_sb[:])
    nc.compile()
    return nc


def _build_neff_b():
    import concourse.tile as tile
    from concourse import bacc, mybir

    nc = bacc.Bacc("TRN2", target_bir_lowering=False, debug=False,
                   num_devices=N_CORES)
    f32, bf = mybir.dt.float32, mybir.dt.bfloat16
    y_hb = nc.dram_tensor("y_hb", [2097152], bf, kind="ExternalInput").ap()
    y_cdb = nc.dram_tensor("y_cdb", [128, 16384], bf, kind="ExternalInput").ap()
    u2 = nc.dram_tensor("u2", [128, 6], bf, kind="ExternalInput").ap()
    v2 = nc.dram_tensor("v2", [128, 96], bf, kind="ExternalInput").ap()
    ta1 = nc.dram_tensor("ta1", [6, 16384], f32, kind="ExternalOutput").ap()
    tc_out = nc.dram_tensor("tc", [48, 8192], f32, kind="ExternalOutput").ap()

    y_hb2 = y_hb.rearrange("(p f) -> p f", p=128)

    with tile.TileContext(nc) as tcx:
        with (
            tcx.tile_pool(name="const", bufs=1) as constp,
            tcx.tile_pool(name="yh", bufs=2) as yhp,
            tcx.tile_pool(name="ycd", bufs=1) as ycdp,
            tcx.tile_pool(name="out", bufs=1) as outp,
            tcx.tile_pool(name="ps1", bufs=4, space="PSUM") as ps1p,
            tcx.tile_pool(name="ps2", bufs=4, space="PSUM") as ps2p,
        ):
            u2_sb = constp.tile([128, 6], bf)
            nc.sync.dma_start(u2_sb[:], u2[:])
            v2_sb = constp.tile([128, 96], bf)
            nc.sync.dma_start(v2_sb[:], v2[:])

            # --- TC path: Kronecker contraction over (c, d_local), bf16
            ycd = []
            for b in range(2):
                t = ycdp.tile([128, 8192], bf, tag=f"ycd{b}")
                nc.sync.dma_start(t[:], y_cdb[:, b * 8192:(b + 1) * 8192])
                ycd.append(t)
            tc_sb = outp.tile([48, 8192], f32, tag="tcsb")
            for k in range(16):
                ps = ps2p.tile([48, 512], f32, tag="ps2")
                nc.tensor.matmul(ps[:], v2_sb[:, 0:48],
                                 ycd[0][:, k * 512:(k + 1) * 512],
                                 start=True, stop=False)
                nc.tensor.matmul(ps[:], v2_sb[:, 48:96],
                                 ycd[1][:, k * 512:(k + 1) * 512],
                                 start=False, stop=True)
                nc.vector.tensor_copy(tc_sb[:, k * 512:(k + 1) * 512], ps[:])
            nc.sync.dma_start(tc_out[:], tc_sb[:])

            # --- TA path: contract h (lhsT = U2 loaded once, 32 big MMs)
            ta1_sb = outp.tile([6, 16384], f32, tag="ta1sb")
            for g in range(4):
                yh = yhp.tile([128, 4096], bf, tag="yh")
                nc.sync.dma_start(yh[:], y_hb2[:, g * 4096:(g + 1) * 4096])
                for m in range(8):
                    ps = ps1p.tile([6, 512], f32, tag="ps1")
                    nc.tensor.matmul(ps[:], u2_sb[:],
                                     yh[:, m * 512:(m + 1) * 512],
                                     start=True, stop=True)
                    off = g * 4096 + m * 512
                    nc.vector.tensor_copy(ta1_sb[:, off:off + 512], ps[:])
            nc.sync.dma_start(ta1[:], ta1_sb[:])
    nc.compile()
    return nc


# ----------------------------------------------------------------- runner
def _make_runner(nc):
    """jit once; inputs may be jax arrays (device-resident) or np arrays.
    Returns dict name -> global jax array ((N_CORES*dim0, ...))."""
    import jax
    import jax.numpy as jnp
    from jax.sharding import Mesh, PartitionSpec, NamedSharding
    from concourse import bass2jax, mybir

    bass2jax.install_neuronx_cc_hook()
    partition_name = (nc.partition_id_tensor.name
                      if nc.partition_id_tensor else None)
    in_names, out_names, out_avals = [], [], []
    for alloc in nc.m.functions[0].allocations:
        if not isinstance(alloc, mybir.MemoryLocationSet):
            continue
        name = alloc.memorylocations[0].name
        if alloc.kind == "ExternalInput":
            if name != partition_name:
                in_names.append(name)
        elif alloc.kind == "ExternalOutput":
            shape = tuple(alloc.tensor_shape)
            dtype = mybir.dt.np(alloc.dtype)
            out_names.append(name)
            out_avals.append(jax.core.ShapedArray(shape, dtype))
    n_params = len(in_names)
    all_names = tuple(in_names + out_names
                      + ([partition_name] if partition_name else []))
    donate = tuple(range(n_params, n_params + len(out_names)))

    def _body(*args):
        operands = list(args)
        if partition_name:
            operands.append(bass2jax.partition_id_tensor())
        outs = bass2jax._bass_exec_p.bind(
            *operands, out_avals=tuple(out_avals), in_names=all_names,
            out_names=tuple(out_names), lowering_input_output_aliases=(),
            sim_require_finite=True, sim_require_nnan=True, nc=nc)
        return tuple(outs)

    devices = jax.devices()[:N_CORES]
    mesh = Mesh(np.asarray(devices), ("core",))
    specs_in = (PartitionSpec("core"),) * (n_params + len(out_names))
    specs_out = (PartitionSpec("core"),) * len(out_names)
    try:
        from jax.experimental.shard_map import shard_map
    except ImportError:
        from jax import shard_map
    fn = jax.jit(shard_map(_body, mesh=mesh, in_specs=specs_in,
                           out_specs=specs_out, check_rep=False),
                 donate_argnums=donate, keep_unused=True)
    shd = NamedSharding(mesh, PartitionSpec("core"))
    zfn = jax.jit(
        lambda: tuple(jnp.zeros((N_CORES * a.shape[0],) + a.shape[1:], a.dtype)
                      for a in out_avals),
        out_shardings=(shd,) * len(out_avals))

    def run(inputs):
        args = [inputs[n] for n in in_names]
        outs = fn(*args, *zfn())
        return dict(zip(out_names, outs))

    run.in_names = in_names
    run.out_names = out_names
    return run


# ----------------------------------------------------------------- host math
def _mode_dot_t(t, u, mode):
    return np.moveaxis(np.tensordot(t, u, axes=(mode, 0)), -1, mode).astype(np.float32)


def _top_evecs(G, rank):
    _, v = np.linalg.eigh(G)
    return np.ascontiguousarray(v[:, ::-1][:, :rank]).astype(np.float32)


def _unfold(t, mode):
    return np.moveaxis(t, mode, 0).reshape(t.shape[mode], -1)


def _factor_inputs(U0, U1, U2, U3):
    u2g = np.tile(U2.astype(BF16), (N_CORES, 1))
    v2g = np.zeros((N_CORES, 128, 96), np.float32)
    for ci in range(N_CORES):
        u1l = U1[ci * DL:(ci + 1) * DL]
        for b in range(2):
            # p = c*4 + dq ; col = b*48 + r0*6 + r1 ; d_local = 2*dq + b
            kr = np.einsum('cr,ds->cdrs', U0, u1l[b::2])
            v2g[ci, :, b * 48:(b + 1) * 48] = kr.reshape(128, 48)
    return u2g, v2g.reshape(N_CORES * 128, 96).astype(BF16)


def _decode(outs):
    """Device outputs -> TA1f (32,64,6,64), TCf (8,6,128,64), both f32."""
    ta1 = np.asarray(outs["ta1"]).reshape(N_CORES, 6, 32, 8, 64)
    tcg = np.asarray(outs["tc"]).reshape(N_CORES, 48, 8192)
    # [ci, r2, c, dl, w] -> [c, ci, dl, r2, w] -> (32, 64, 6, 64)
    TA1f = ta1.transpose(2, 0, 3, 1, 4).reshape(32, 64, 6, 64)
    TCf = tcg.sum(axis=0, dtype=np.float32).reshape(8, 6, 128, 64)
    return np.ascontiguousarray(TA1f), TCf


# ----------------------------------------------------------------- kernel
def kernel(x, conv_w, conv_b, fc_w, fc_b):
    x = np.asarray(x, np.float32)
    conv_w = np.asarray(conv_w, np.float32)
    conv_b = np.asarray(conv_b, np.float32)
    fc_w = np.asarray(fc_w, np.float32)
    fc_b = np.asarray(fc_b, np.float32)

    if "runA" not in _cache:
        _cache["ncA"] = _build_neff_a()
        _cache["ncB"] = _build_neff_b()
        _cache["runA"] = _make_runner(_cache["ncA"])
        _cache["runB"] = _make_runner(_cache["ncB"])
    runA, runB = _cache["runA"], _cache["runB"]

    # host-side input staging ------------------------------------------------
    # x_im[ci, q*48 + dq*12 + kh*4 + i, h*64+w] = x[i, 8ci+2dq+q, h+kh, w]
    x_im = np.empty((N_CORES, 96, 8192), np.float32)
    for ci in range(N_CORES):
        for q in range(2):
            for dq in range(4):
                d = 8 * ci + 2 * dq + q
                for kh in range(3):
                    r0 = q * 48 + dq * 12 + kh * 4
                    x_im[ci, r0:r0 + 4, :] = \
                        x[:, d, kh:kh + 128, :].reshape(4, 8192)
    w48 = np.zeros((48, 128), np.float32)
    for dq in range(4):
        for kh in range(3):
            for i in range(4):
                w48[dq * 12 + kh * 4 + i, dq::4] = conv_w[:, i, 0, kh, 0]
    b128 = np.repeat(conv_b, 4).reshape(128, 1).astype(np.float32)

    inA = {
        "x_im": x_im.reshape(N_CORES * 96, 8192),
        "w48": np.tile(w48, (N_CORES, 1)),
        "b128": np.tile(b128, (N_CORES, 1)),
    }
    stash["inA"] = inA
    outsA = runA(inA)
    y_hb_g, y_cdb_g = outsA["y_hb"], outsA["y_cdb"]
    stash["y_hb"] = y_hb_g
    stash["y_cdb"] = y_cdb_g

    # Jacobi-HOOI, identity init, host eigh ---------------------------------
    U0 = np.eye(C, dtype=np.float32)[:, :RANKS[0]]
    U1 = np.eye(D, dtype=np.float32)[:, :RANKS[1]]
    U2 = np.eye(H, dtype=np.float32)[:, :RANKS[2]]
    U3 = np.eye(W, dtype=np.float32)[:, :RANKS[3]]

    for sweep in range(N_SWEEPS + 1):
        u2g, v2g = _factor_inputs(U0, U1, U2, U3)
        inB = {"y_hb": y_hb_g, "y_cdb": y_cdb_g, "u2": u2g, "v2": v2g}
        if sweep == 0:
            stash["inB_small"] = {"u2": u2g, "v2": v2g}
        outs = runB(inB)
        TA1f, TCf = _decode(outs)
        TA2f = np.tensordot(TA1f, U3, axes=(3, 0))        # (32, 64, 6, 6)
        P0 = _mode_dot_t(TA2f, U1, 1)                     # (32, 6, 6, 6)
        if sweep == N_SWEEPS:
            core = np.tensordot(U0, P0, axes=(0, 0))      # (8, 6, 6, 6)
            break
        P1 = np.moveaxis(np.tensordot(TA2f, U0, axes=(0, 0)), -1, 0)
        P2 = _mode_dot_t(TCf, U3, 3)                      # (8, 6, 128, 6)
        P3 = _mode_dot_t(TCf, U2, 2)                      # (8, 6, 6, 64)
        newU = []
        for mode, P in ((0, P0), (1, P1), (2, P2), (3, P3)):
            M = _unfold(P, mode)
            G = (M @ M.T).astype(np.float32)
            newU.append(_top_evecs(G, RANKS[mode]))
        U0, U1, U2, U3 = newU

    logits = core.reshape(-1) @ fc_w.T + fc_b
    e = np.exp(logits - logits.max())
    return (e / e.sum()).astype(np.float32)


# revision 6
# speedup vs baseline: 1.8584x; 1.1752x over previous
"""Trainium2 Bass kernel for nn_ActorNetwork (conv3d + Tucker/HOOI + FC + softmax).

Structure (8 NeuronCores, D-dim sharded 64 -> 8 per core):
  NEFF-A (1 launch): conv3d(k=(1,3,1)) + bias + relu via TensorE matmul
      (K=48 block-diag weights -> 128 output partitions, fp32), cast to
      bf16, write y in two DRAM layouts:
      y_hb [h=128, (c,d,w)] and y_cdb [(c,dq)=128, q, (h,w)], d = 2*dq+q.
  NEFF-B (N_SWEEPS+1 launches): one Jacobi-HOOI sweep's device work
      (bf16 matmuls, fp32 PSUM):
      TA1 = y x2 U2   [6, (c,d,w)] per-core local-d block (contract h)
      TC  = y x0 U0 x1 U1l  [48, (h,w)] partial sum over local d (Kronecker)
  Host between launches: w-contraction of TA1, projections, small Grams,
      LAPACK eigh (host LAPACK required: eigenvector signs must match the
      CPU reference; the neuron backend cannot lower eigh at all).
  Final: core tensor from last launch, FC + softmax on host.

y stays device-resident between launches as jax arrays (custom PJRT runner).
"""

import os
import numpy as np
import ml_dtypes

BF16 = ml_dtypes.bfloat16
N_CORES = 8
RANKS = (8, 6, 6, 6)
N_SWEEPS = int(os.environ.get("KERNEL_N_SWEEPS", "12"))
C, D, H, W = 32, 64, 128, 64  # y dims; x is (4, 64, 130, 64)
DL = D // N_CORES

_cache = {}
stash = {}


# ----------------------------------------------------------------- builders
def _build_neff_a():
    import concourse.tile as tile
    from concourse import bacc, mybir

    nc = bacc.Bacc("TRN2", target_bir_lowering=False, debug=False,
                   num_devices=N_CORES)
    f32, bf = mybir.dt.float32, mybir.dt.bfloat16
    x_im = nc.dram_tensor("x_im", [96, 8192], f32, kind="ExternalInput").ap()
    w48 = nc.dram_tensor("w48", [48, 128], f32, kind="ExternalInput").ap()
    b128 = nc.dram_tensor("b128", [128, 1], f32, kind="ExternalInput").ap()
    y_hb = nc.dram_tensor("y_hb", [2097152], bf, kind="ExternalOutput").ap()
    y_cdb = nc.dram_tensor("y_cdb", [128, 16384], bf, kind="ExternalOutput").ap()

    # y_hb flat = h*16384 + c*512 + d*64 + w ; d = 2*dq + q ; p = c*4+dq
    y_hb_v = y_hb.rearrange("(h cdq q w) -> q cdq h w",
                            h=128, cdq=128, q=2, w=64)

    with tile.TileContext(nc) as tcx:
        with (
            tcx.tile_pool(name="const", bufs=1) as constp,
            tcx.tile_pool(name="imc", bufs=2) as imcp,
            tcx.tile_pool(name="ysb", bufs=2) as ysbp,
            tcx.tile_pool(name="ps", bufs=8, space="PSUM") as psp,
        ):
            w_sb = constp.tile([48, 128], f32)
            nc.sync.dma_start(w_sb[:], w48[:])
            b_sb = constp.tile([128, 1], f32)
            nc.sync.dma_start(b_sb[:], b128[:])

            for q in range(2):
                imc = imcp.tile([48, 8192], f32, tag="imc")
                nc.sync.dma_start(imc[:], x_im[q * 48:(q + 1) * 48, :])
                y_sb = ysbp.tile([128, 8192], bf, tag="ysb")
                for n in range(16):
                    ps = psp.tile([128, 512], f32, tag="ps")
                    nc.tensor.matmul(ps[:], w_sb[:], imc[:, n * 512:(n + 1) * 512],
                                     start=True, stop=True)
                    nc.scalar.activation(y_sb[:, n * 512:(n + 1) * 512], ps[:],
                                         mybir.ActivationFunctionType.Relu,
                                         bias=b_sb[:])
                nc.sync.dma_start(y_cdb[:, q * 8192:(q + 1) * 8192], y_sb[:])
                nc.sync.dma_start(y_hb_v[q], y_sb[:])
    nc.compile()
    return nc


def _build_neff_b():
    import concourse.tile as tile
    from concourse import bacc, mybir

    nc = bacc.Bacc("TRN2", target_bir_lowering=False, debug=False,
                   num_devices=N_CORES)
    f32, bf = mybir.dt.float32, mybir.dt.bfloat16
    y_hb = nc.dram_tensor("y_hb", [2097152], bf, kind="ExternalInput").ap()
    y_cdb = nc.dram_tensor("y_cdb", [128, 16384], bf, kind="ExternalInput").ap()
    u2 = nc.dram_tensor("u2", [128, 6], bf, kind="ExternalInput").ap()
    v2 = nc.dram_tensor("v2", [128, 96], bf, kind="ExternalInput").ap()
    ta1 = nc.dram_tensor("ta1", [6, 16384], f32, kind="ExternalOutput").ap()
    tc_out = nc.dram_tensor("tc", [48, 8192], f32, kind="ExternalOutput").ap()

    y_hb2 = y_hb.rearrange("(p f) -> p f", p=128)

    with tile.TileContext(nc) as tcx:
        with (
            tcx.tile_pool(name="const", bufs=1) as constp,
            tcx.tile_pool(name="yh", bufs=2) as yhp,
            tcx.tile_pool(name="ycd", bufs=1) as ycdp,
            tcx.tile_pool(name="out", bufs=1) as outp,
            tcx.tile_pool(name="ps1", bufs=4, space="PSUM") as ps1p,
            tcx.tile_pool(name="ps2", bufs=4, space="PSUM") as ps2p,
        ):
            u2_sb = constp.tile([128, 6], bf)
            nc.sync.dma_start(u2_sb[:], u2[:])
            v2_sb = constp.tile([128, 96], bf)
            nc.sync.dma_start(v2_sb[:], v2[:])

            # Interleaved chunked loads: per group g (quarter of columns),
            # load yh[g], ycd0[g], ycd1[g]; compute as chunks arrive; write
            # outputs per-group so output DMA overlaps later groups.
            tc_sb = outp.tile([48, 8192], f32, tag="tcsb")
            ta1_sb = outp.tile([6, 16384], f32, tag="ta1sb")
            ycd0 = ycdp.tile([128, 8192], bf, tag="ycd0")
            ycd1 = ycdp.tile([128, 8192], bf, tag="ycd1")
            for g in range(4):
                yh = yhp.tile([128, 4096], bf, tag="yh")
                nc.sync.dma_start(yh[:], y_hb2[:, g * 4096:(g + 1) * 4096])
                c0 = ycd0[:, g * 2048:(g + 1) * 2048]
                nc.sync.dma_start(c0, y_cdb[:, g * 2048:(g + 1) * 2048])
                c1 = ycd1[:, g * 2048:(g + 1) * 2048]
                nc.sync.dma_start(c1, y_cdb[:, 8192 + g * 2048:8192 + (g + 1) * 2048])
                # TA: contract h; copies on Scalar (ACT) engine
                for m in range(8):
                    ps = ps1p.tile([6, 512], f32, tag="ps1")
                    nc.tensor.matmul(ps[:], u2_sb[:],
                                     yh[:, m * 512:(m + 1) * 512],
                                     start=True, stop=True)
                    off = g * 4096 + m * 512
                    nc.scalar.copy(ta1_sb[:, off:off + 512], ps[:])
                # TC: Kronecker contraction; copies on Vector (DVE) engine
                for kk in range(4):
                    k = g * 4 + kk
                    ps = ps2p.tile([48, 512], f32, tag="ps2")
                    nc.tensor.matmul(ps[:], v2_sb[:, 0:48],
                                     ycd0[:, k * 512:(k + 1) * 512],
                                     start=True, stop=False)
                    nc.tensor.matmul(ps[:], v2_sb[:, 48:96],
                                     ycd1[:, k * 512:(k + 1) * 512],
                                     start=False, stop=True)
                    nc.vector.tensor_copy(tc_sb[:, k * 512:(k + 1) * 512], ps[:])
                nc.sync.dma_start(tc_out[:, g * 2048:(g + 1) * 2048],
                                  tc_sb[:, g * 2048:(g + 1) * 2048])
                nc.sync.dma_start(ta1[:, g * 4096:(g + 1) * 4096],
                                  ta1_sb[:, g * 4096:(g + 1) * 4096])
    nc.compile()
    return nc


# ----------------------------------------------------------------- runner
def _make_runner(nc):
    """jit once; inputs may be jax arrays (device-resident) or np arrays.
    Returns dict name -> global jax array ((N_CORES*dim0, ...))."""
    import jax
    import jax.numpy as jnp
    from jax.sharding import Mesh, PartitionSpec, NamedSharding
    from concourse import bass2jax, mybir

    bass2jax.install_neuronx_cc_hook()
    partition_name = (nc.partition_id_tensor.name
                      if nc.partition_id_tensor else None)
    in_names, out_names, out_avals = [], [], []
    for alloc in nc.m.functions[0].allocations:
        if not isinstance(alloc, mybir.MemoryLocationSet):
            continue
        name = alloc.memorylocations[0].name
        if alloc.kind == "ExternalInput":
            if name != partition_name:
                in_names.append(name)
        elif alloc.kind == "ExternalOutput":
            shape = tuple(alloc.tensor_shape)
            dtype = mybir.dt.np(alloc.dtype)
            out_names.append(name)
            out_avals.append(jax.core.ShapedArray(shape, dtype))
    n_params = len(in_names)
    all_names = tuple(in_names + out_names
                      + ([partition_name] if partition_name else []))
    donate = tuple(range(n_params, n_params + len(out_names)))

    def _body(*args):
        operands = list(args)
        if partition_name:
            operands.append(bass2jax.partition_id_tensor())
        outs = bass2jax._bass_exec_p.bind(
            *operands, out_avals=tuple(out_avals), in_names=all_names,
            out_names=tuple(out_names), lowering_input_output_aliases=(),
            sim_require_finite=True, sim_require_nnan=True, nc=nc)
        return tuple(outs)

    devices = jax.devices()[:N_CORES]
    mesh = Mesh(np.asarray(devices), ("core",))
    specs_in = (PartitionSpec("core"),) * (n_params + len(out_names))
    specs_out = (PartitionSpec("core"),) * len(out_names)
    try:
        from jax.experimental.shard_map import shard_map
    except ImportError:
        from jax import shard_map
    fn = jax.jit(shard_map(_body, mesh=mesh, in_specs=specs_in,
                           out_specs=specs_out, check_rep=False),
                 donate_argnums=donate, keep_unused=True)
    shd = NamedSharding(mesh, PartitionSpec("core"))
    zfn = jax.jit(
        lambda: tuple(jnp.zeros((N_CORES * a.shape[0],) + a.shape[1:], a.dtype)
                      for a in out_avals),
        out_shardings=(shd,) * len(out_avals))

    def run(inputs):
        args = [inputs[n] for n in in_names]
        outs = fn(*args, *zfn())
        return dict(zip(out_names, outs))

    run.in_names = in_names
    run.out_names = out_names
    return run


# ----------------------------------------------------------------- host math
def _mode_dot_t(t, u, mode):
    return np.moveaxis(np.tensordot(t, u, axes=(mode, 0)), -1, mode).astype(np.float32)


def _top_evecs(G, rank):
    _, v = np.linalg.eigh(G)
    return np.ascontiguousarray(v[:, ::-1][:, :rank]).astype(np.float32)


def _unfold(t, mode):
    return np.moveaxis(t, mode, 0).reshape(t.shape[mode], -1)


def _factor_inputs(U0, U1, U2, U3):
    u2g = np.tile(U2.astype(BF16), (N_CORES, 1))
    v2g = np.zeros((N_CORES, 128, 96), np.float32)
    for ci in range(N_CORES):
        u1l = U1[ci * DL:(ci + 1) * DL]
        for b in range(2):
            # p = c*4 + dq ; col = b*48 + r0*6 + r1 ; d_local = 2*dq + b
            kr = np.einsum('cr,ds->cdrs', U0, u1l[b::2])
            v2g[ci, :, b * 48:(b + 1) * 48] = kr.reshape(128, 48)
    return u2g, v2g.reshape(N_CORES * 128, 96).astype(BF16)


def _decode(outs):
    """Device outputs -> TA1f (32,64,6,64), TCf (8,6,128,64), both f32."""
    ta1 = np.asarray(outs["ta1"]).reshape(N_CORES, 6, 32, 8, 64)
    tcg = np.asarray(outs["tc"]).reshape(N_CORES, 48, 8192)
    # [ci, r2, c, dl, w] -> [c, ci, dl, r2, w] -> (32, 64, 6, 64)
    TA1f = ta1.transpose(2, 0, 3, 1, 4).reshape(32, 64, 6, 64)
    TCf = tcg.sum(axis=0, dtype=np.float32).reshape(8, 6, 128, 64)
    return np.ascontiguousarray(TA1f), TCf


# ----------------------------------------------------------------- kernel
def kernel(x, conv_w, conv_b, fc_w, fc_b):
    x = np.asarray(x, np.float32)
    conv_w = np.asarray(conv_w, np.float32)
    conv_b = np.asarray(conv_b, np.float32)
    fc_w = np.asarray(fc_w, np.float32)
    fc_b = np.asarray(fc_b, np.float32)

    if "runA" not in _cache:
        _cache["ncA"] = _build_neff_a()
        _cache["ncB"] = _build_neff_b()
        _cache["runA"] = _make_runner(_cache["ncA"])
        _cache["runB"] = _make_runner(_cache["ncB"])
    runA, runB = _cache["runA"], _cache["runB"]

    # host-side input staging ------------------------------------------------
    # x_im[ci, q*48 + dq*12 + kh*4 + i, h*64+w] = x[i, 8ci+2dq+q, h+kh, w]
    x_im = np.empty((N_CORES, 96, 8192), np.float32)
    for ci in range(N_CORES):
        for q in range(2):
            for dq in range(4):
                d = 8 * ci + 2 * dq + q
                for kh in range(3):
                    r0 = q * 48 + dq * 12 + kh * 4
                    x_im[ci, r0:r0 + 4, :] = \
                        x[:, d, kh:kh + 128, :].reshape(4, 8192)
    w48 = np.zeros((48, 128), np.float32)
    for dq in range(4):
        for kh in range(3):
            for i in range(4):
                w48[dq * 12 + kh * 4 + i, dq::4] = conv_w[:, i, 0, kh, 0]
    b128 = np.repeat(conv_b, 4).reshape(128, 1).astype(np.float32)

    inA = {
        "x_im": x_im.reshape(N_CORES * 96, 8192),
        "w48": np.tile(w48, (N_CORES, 1)),
        "b128": np.tile(b128, (N_CORES, 1)),
    }
    stash["inA"] = inA
    outsA = runA(inA)
    y_hb_g, y_cdb_g = outsA["y_hb"], outsA["y_cdb"]
    stash["y_hb"] = y_hb_g
    stash["y_cdb"] = y_cdb_g

    # Jacobi-HOOI, identity init, host eigh ---------------------------------
    U0 = np.eye(C, dtype=np.float32)[:, :RANKS[0]]
    U1 = np.eye(D, dtype=np.float32)[:, :RANKS[1]]
    U2 = np.eye(H, dtype=np.float32)[:, :RANKS[2]]
    U3 = np.eye(W, dtype=np.float32)[:, :RANKS[3]]

    for sweep in range(N_SWEEPS + 1):
        u2g, v2g = _factor_inputs(U0, U1, U2, U3)
        inB = {"y_hb": y_hb_g, "y_cdb": y_cdb_g, "u2": u2g, "v2": v2g}
        if sweep == 0:
            stash["inB_small"] = {"u2": u2g, "v2": v2g}
        outs = runB(inB)
        TA1f, TCf = _decode(outs)
        TA2f = np.tensordot(TA1f, U3, axes=(3, 0))        # (32, 64, 6, 6)
        P0 = _mode_dot_t(TA2f, U1, 1)                     # (32, 6, 6, 6)
        if sweep == N_SWEEPS:
            core = np.tensordot(U0, P0, axes=(0, 0))      # (8, 6, 6, 6)
            break
        P1 = np.moveaxis(np.tensordot(TA2f, U0, axes=(0, 0)), -1, 0)
        P2 = _mode_dot_t(TCf, U3, 3)                      # (8, 6, 128, 6)
        P3 = _mode_dot_t(TCf, U2, 2)                      # (8, 6, 6, 64)
        newU = []
        for mode, P in ((0, P0), (1, P1), (2, P2), (3, P3)):
            M = _unfold(P, mode)
            G = (M @ M.T).astype(np.float32)
            newU.append(_top_evecs(G, RANKS[mode]))
        U0, U1, U2, U3 = newU

    logits = core.reshape(-1) @ fc_w.T + fc_b
    e = np.exp(logits - logits.max())
    return (e / e.sum()).astype(np.float32)


# revision 11
# speedup vs baseline: 5.5189x; 2.9697x over previous
"""Trainium2 Bass kernel for nn_ActorNetwork (conv3d + Tucker/HOOI + FC + softmax).

Structure (8 NeuronCores, D-dim sharded 64 -> 8 per core):
  NEFF-A (1 launch): conv3d(k=(1,3,1)) + bias + relu via TensorE matmul
      (K=48 block-diag weights -> 128 output partitions, fp32), cast to
      bf16, write y in two DRAM layouts:
      y_hb [h=128, (c,d,w)] and y_cdb [(c,dq)=128, q, (h,w)], d = 2*dq+q.
  NEFF-B (N_SWEEPS+1 launches): one Jacobi-HOOI sweep's device work
      (bf16 matmuls, fp32 PSUM):
      TA1 = y x2 U2   [6, (c,d,w)] per-core local-d block (contract h)
      TC  = y x0 U0 x1 U1l  [48, (h,w)] partial sum over local d (Kronecker)
  Host between launches: w-contraction of TA1, projections, small Grams,
      LAPACK eigh (host LAPACK required: eigenvector signs must match the
      CPU reference; the neuron backend cannot lower eigh at all).
  Final: core tensor from last launch, FC + softmax on host.

y stays device-resident between launches as jax arrays (custom PJRT runner).
"""

import os
import numpy as np
import ml_dtypes

BF16 = ml_dtypes.bfloat16
N_CORES = 8
RANKS = (8, 6, 6, 6)
N_SWEEPS = int(os.environ.get("KERNEL_N_SWEEPS", "12"))
C, D, H, W = 32, 64, 128, 64  # y dims; x is (4, 64, 130, 64)
DL = D // N_CORES

_cache = {}
stash = {}


# ----------------------------------------------------------------- builders
def _build_neff_a():
    import concourse.tile as tile
    from concourse import bacc, mybir

    nc = bacc.Bacc("TRN2", target_bir_lowering=False, debug=False,
                   num_devices=N_CORES)
    f32, bf = mybir.dt.float32, mybir.dt.bfloat16
    x_im = nc.dram_tensor("x_im", [96, 8192], bf, kind="ExternalInput").ap()
    w48 = nc.dram_tensor("w48", [48, 128], bf, kind="ExternalInput").ap()
    b128 = nc.dram_tensor("b128", [128, 1], f32, kind="ExternalInput").ap()
    y_hb = nc.dram_tensor("y_hb", [2097152], bf, kind="ExternalOutput").ap()
    y_cdb = nc.dram_tensor("y_cdb", [128, 16384], bf, kind="ExternalOutput").ap()

    # y_hb flat = h*16384 + c*512 + d*64 + w ; d = 2*dq + q ; p = c*4+dq
    y_hb_v = y_hb.rearrange("(h cdq q w) -> q cdq h w",
                            h=128, cdq=128, q=2, w=64)

    with tile.TileContext(nc) as tcx:
        with (
            tcx.tile_pool(name="const", bufs=1) as constp,
            tcx.tile_pool(name="imc", bufs=1) as imcp,
            tcx.tile_pool(name="ysb", bufs=2) as ysbp,
            tcx.tile_pool(name="ps", bufs=8, space="PSUM") as psp,
        ):
            w_sb = constp.tile([128, 128], bf)
            nc.sync.dma_start(w_sb[0:48, :], w48[:])
            nc.sync.dma_start(w_sb[64:112, :], w48[:])
            b_sb = constp.tile([128, 1], f32)
            nc.sync.dma_start(b_sb[:], b128[:])

            imc = imcp.tile([128, 8192], bf, tag="imc")
            for g in range(2):
                nc.sync.dma_start(imc[0:48, g * 4096:(g + 1) * 4096],
                                  x_im[0:48, g * 4096:(g + 1) * 4096])
                nc.sync.dma_start(imc[64:112, g * 4096:(g + 1) * 4096],
                                  x_im[48:96, g * 4096:(g + 1) * 4096])
            for q in range(2):
                y_sb = ysbp.tile([128, 8192], bf, tag="ysb")
                for n in range(16):
                    ps = psp.tile([128, 512], f32, tag="ps")
                    nc.tensor.matmul(ps[:], w_sb[64 * q:64 * q + 48, :],
                                     imc[64 * q:64 * q + 48,
                                         n * 512:(n + 1) * 512],
                                     start=True, stop=True)
                    nc.scalar.activation(y_sb[:, n * 512:(n + 1) * 512], ps[:],
                                         mybir.ActivationFunctionType.Relu,
                                         bias=b_sb[:])
                nc.scalar.dma_start(y_cdb[:, q * 8192:(q + 1) * 8192], y_sb[:])
                nc.sync.dma_start(y_hb_v[q], y_sb[:])
    nc.compile()
    return nc


def _build_neff_b():
    import concourse.tile as tile
    from concourse import bacc, mybir

    nc = bacc.Bacc("TRN2", target_bir_lowering=False, debug=False,
                   num_devices=N_CORES)
    f32, bf = mybir.dt.float32, mybir.dt.bfloat16
    y_hb = nc.dram_tensor("y_hb", [2097152], bf, kind="ExternalInput").ap()
    y_cdb = nc.dram_tensor("y_cdb", [128, 16384], bf, kind="ExternalInput").ap()
    u2 = nc.dram_tensor("u2", [128, 6], bf, kind="ExternalInput").ap()
    v2 = nc.dram_tensor("v2", [128, 96], bf, kind="ExternalInput").ap()
    ta1 = nc.dram_tensor("ta1", [6, 16384], bf, kind="ExternalOutput").ap()
    tc_out = nc.dram_tensor("tc", [112, 4096], bf, kind="ExternalOutput").ap()

    y_hb2 = y_hb.rearrange("(p f) -> p f", p=128)

    with tile.TileContext(nc) as tcx:
        with (
            tcx.tile_pool(name="const", bufs=1) as constp,
            tcx.tile_pool(name="yh", bufs=2) as yhp,
            tcx.tile_pool(name="ycd", bufs=1) as ycdp,
            tcx.tile_pool(name="out", bufs=1) as outp,
            tcx.tile_pool(name="ps1", bufs=4, space="PSUM") as ps1p,
            tcx.tile_pool(name="ps2", bufs=4, space="PSUM") as ps2p,
        ):
            u2_sb = constp.tile([128, 6], bf)
            nc.sync.dma_start(u2_sb[:], u2[:])
            v2_sb = constp.tile([128, 96], bf)
            nc.sync.dma_start(v2_sb[:], v2[:])

            # 2 column-groups; loads on sync queue, stores on scalar queue
            # (separate HWDGE queues avoid FIFO head-of-line blocking).
            tc_sb = outp.tile([112, 4096], bf, tag="tcsb")
            ta1_sb = outp.tile([6, 16384], bf, tag="ta1sb")
            ycd0 = ycdp.tile([128, 8192], bf, tag="ycd0")
            ycd1 = ycdp.tile([128, 8192], bf, tag="ycd1")
            for g in range(2):
                yh = yhp.tile([128, 8192], bf, tag="yh")
                nc.sync.dma_start(yh[:], y_hb2[:, g * 8192:(g + 1) * 8192])
                c0 = ycd0[:, g * 4096:(g + 1) * 4096]
                nc.sync.dma_start(c0, y_cdb[:, g * 4096:(g + 1) * 4096])
                c1 = ycd1[:, g * 4096:(g + 1) * 4096]
                nc.sync.dma_start(c1, y_cdb[:, 8192 + g * 4096:8192 + (g + 1) * 4096])
                # TA: contract h; copies on Scalar (ACT) engine
                for m in range(16):
                    ps = ps1p.tile([6, 512], f32, tag="ps1")
                    nc.tensor.matmul(ps[:], u2_sb[:],
                                     yh[:, m * 512:(m + 1) * 512],
                                     start=True, stop=True)
                    off = g * 8192 + m * 512
                    nc.scalar.copy(ta1_sb[:, off:off + 512], ps[:])
                # TC: Kronecker contraction; copies on Vector (DVE) engine
                for kk in range(8):
                    k = g * 8 + kk
                    ps = ps2p.tile([48, 512], f32, tag="ps2")
                    nc.tensor.matmul(ps[:], v2_sb[:, 0:48],
                                     ycd0[:, k * 512:(k + 1) * 512],
                                     start=True, stop=False)
                    nc.tensor.matmul(ps[:], v2_sb[:, 48:96],
                                     ycd1[:, k * 512:(k + 1) * 512],
                                     start=False, stop=True)
                    nc.vector.tensor_copy(
                        tc_sb[64 * (k % 2):64 * (k % 2) + 48,
                              (k // 2) * 512:(k // 2 + 1) * 512], ps[:])
                nc.scalar.dma_start(tc_out[:, 4 * g * 512:(4 * g + 4) * 512],
                                    tc_sb[:, 4 * g * 512:(4 * g + 4) * 512])
                nc.scalar.dma_start(ta1[:, g * 8192:(g + 1) * 8192],
                                    ta1_sb[:, g * 8192:(g + 1) * 8192])
    nc.compile()
    return nc


# ----------------------------------------------------------------- runner
def _make_runner(nc):
    """jit once; inputs may be jax arrays (device-resident) or np arrays.
    Returns dict name -> global jax array ((N_CORES*dim0, ...))."""
    import jax
    import jax.numpy as jnp
    from jax.sharding import Mesh, PartitionSpec, NamedSharding
    from concourse import bass2jax, mybir

    bass2jax.install_neuronx_cc_hook()
    partition_name = (nc.partition_id_tensor.name
                      if nc.partition_id_tensor else None)
    in_names, out_names, out_avals = [], [], []
    for alloc in nc.m.functions[0].allocations:
        if not isinstance(alloc, mybir.MemoryLocationSet):
            continue
        name = alloc.memorylocations[0].name
        if alloc.kind == "ExternalInput":
            if name != partition_name:
                in_names.append(name)
        elif alloc.kind == "ExternalOutput":
            shape = tuple(alloc.tensor_shape)
            dtype = mybir.dt.np(alloc.dtype)
            out_names.append(name)
            out_avals.append(jax.core.ShapedArray(shape, dtype))
    n_params = len(in_names)
    all_names = tuple(in_names + out_names
                      + ([partition_name] if partition_name else []))
    donate = tuple(range(n_params, n_params + len(out_names)))

    def _body(*args):
        operands = list(args)
        if partition_name:
            operands.append(bass2jax.partition_id_tensor())
        outs = bass2jax._bass_exec_p.bind(
            *operands, out_avals=tuple(out_avals), in_names=all_names,
            out_names=tuple(out_names), lowering_input_output_aliases=(),
            sim_require_finite=True, sim_require_nnan=True, nc=nc)
        return tuple(outs)

    devices = jax.devices()[:N_CORES]
    mesh = Mesh(np.asarray(devices), ("core",))
    specs_in = (PartitionSpec("core"),) * (n_params + len(out_names))
    specs_out = (PartitionSpec("core"),) * len(out_names)
    try:
        from jax.experimental.shard_map import shard_map
    except ImportError:
        from jax import shard_map
    fn = jax.jit(shard_map(_body, mesh=mesh, in_specs=specs_in,
                           out_specs=specs_out, check_rep=False),
                 donate_argnums=donate, keep_unused=True)
    shd = NamedSharding(mesh, PartitionSpec("core"))
    zfn = jax.jit(
        lambda: tuple(jnp.zeros((N_CORES * a.shape[0],) + a.shape[1:], a.dtype)
                      for a in out_avals),
        out_shardings=(shd,) * len(out_avals))

    def run(inputs):
        args = [inputs[n] for n in in_names]
        outs = fn(*args, *zfn())
        return dict(zip(out_names, outs))

    run.in_names = in_names
    run.out_names = out_names
    return run


# ----------------------------------------------------------------- host math
def _mode_dot_t(t, u, mode):
    return np.moveaxis(np.tensordot(t, u, axes=(mode, 0)), -1, mode).astype(np.float32)


def _top_evecs(G, rank):
    _, v = np.linalg.eigh(G)
    return np.ascontiguousarray(v[:, ::-1][:, :rank]).astype(np.float32)


def _unfold(t, mode):
    return np.moveaxis(t, mode, 0).reshape(t.shape[mode], -1)


def _factor_inputs(U0, U1, U2, U3):
    u2g = np.tile(U2.astype(BF16), (N_CORES, 1))
    v2g = np.zeros((N_CORES, 128, 96), np.float32)
    for ci in range(N_CORES):
        u1l = U1[ci * DL:(ci + 1) * DL]
        for b in range(2):
            # p = c*4 + dq ; col = b*48 + r0*6 + r1 ; d_local = 2*dq + b
            kr = np.einsum('cr,ds->cdrs', U0, u1l[b::2])
            v2g[ci, :, b * 48:(b + 1) * 48] = kr.reshape(128, 48)
    return u2g, v2g.reshape(N_CORES * 128, 96).astype(BF16)


def _decode(outs):
    """Device outputs -> TA1f (32,64,6,64), TCf (8,6,128,64), both f32."""
    ta1 = np.asarray(outs["ta1"]).astype(np.float32) \
            .reshape(N_CORES, 6, 32, 8, 64)            # [ci, r2, c, dl, w]
    TA1f = ta1.transpose(2, 0, 3, 1, 4).reshape(32, 64, 6, 64)
    tcw = np.asarray(outs["tc"]).astype(np.float32) \
            .reshape(N_CORES, 112, 8, 512)
    tcg = np.stack([tcw[:, 0:48], tcw[:, 64:112]], axis=3) \
            .reshape(N_CORES, 48, 8192)                # k = kd*2 + km
    TCf = tcg.sum(axis=0, dtype=np.float32).reshape(8, 6, 128, 64)
    return np.ascontiguousarray(TA1f), TCf


# ----------------------------------------------------------------- kernel
def kernel(x, conv_w, conv_b, fc_w, fc_b):
    x = np.asarray(x, np.float32)
    conv_w = np.asarray(conv_w, np.float32)
    conv_b = np.asarray(conv_b, np.float32)
    fc_w = np.asarray(fc_w, np.float32)
    fc_b = np.asarray(fc_b, np.float32)

    if "runA" not in _cache:
        _cache["ncA"] = _build_neff_a()
        _cache["ncB"] = _build_neff_b()
        _cache["runA"] = _make_runner(_cache["ncA"])
        _cache["runB"] = _make_runner(_cache["ncB"])
    runA, runB = _cache["runA"], _cache["runB"]

    # host-side input staging ------------------------------------------------
    # x_im[ci, q*48 + dq*12 + kh*4 + i, h*64+w] = x[i, 8ci+2dq+q, h+kh, w]
    x_im = np.empty((N_CORES, 96, 8192), np.float32)
    for ci in range(N_CORES):
        for q in range(2):
            for dq in range(4):
                d = 8 * ci + 2 * dq + q
                for kh in range(3):
                    r0 = q * 48 + dq * 12 + kh * 4
                    x_im[ci, r0:r0 + 4, :] = \
                        x[:, d, kh:kh + 128, :].reshape(4, 8192)
    w48 = np.zeros((48, 128), np.float32)
    for dq in range(4):
        for kh in range(3):
            for i in range(4):
                w48[dq * 12 + kh * 4 + i, dq::4] = conv_w[:, i, 0, kh, 0]
    b128 = np.repeat(conv_b, 4).reshape(128, 1).astype(np.float32)

    inA = {
        "x_im": x_im.reshape(N_CORES * 96, 8192).astype(BF16),
        "w48": np.tile(w48, (N_CORES, 1)).astype(BF16),
        "b128": np.tile(b128, (N_CORES, 1)),
    }
    stash["inA"] = inA
    outsA = runA(inA)
    y_hb_g, y_cdb_g = outsA["y_hb"], outsA["y_cdb"]
    stash["y_hb"] = y_hb_g
    stash["y_cdb"] = y_cdb_g

    # Jacobi-HOOI, identity init, host eigh ---------------------------------
    U0 = np.eye(C, dtype=np.float32)[:, :RANKS[0]]
    U1 = np.eye(D, dtype=np.float32)[:, :RANKS[1]]
    U2 = np.eye(H, dtype=np.float32)[:, :RANKS[2]]
    U3 = np.eye(W, dtype=np.float32)[:, :RANKS[3]]

    for sweep in range(N_SWEEPS + 1):
        u2g, v2g = _factor_inputs(U0, U1, U2, U3)
        inB = {"y_hb": y_hb_g, "y_cdb": y_cdb_g, "u2": u2g, "v2": v2g}
        if sweep == 0:
            stash["inB_small"] = {"u2": u2g, "v2": v2g}
        outs = runB(inB)
        TA1f, TCf = _decode(outs)
        TA2f = np.tensordot(TA1f, U3, axes=(3, 0))        # (32, 64, 6, 6)
        P0 = _mode_dot_t(TA2f, U1, 1)                     # (32, 6, 6, 6)
        if sweep == N_SWEEPS:
            core = np.tensordot(U0, P0, axes=(0, 0))      # (8, 6, 6, 6)
            break
        P1 = np.moveaxis(np.tensordot(TA2f, U0, axes=(0, 0)), -1, 0)
        P2 = _mode_dot_t(TCf, U3, 3)                      # (8, 6, 128, 6)
        P3 = _mode_dot_t(TCf, U2, 2)                      # (8, 6, 6, 64)
        newU = []
        for mode, P in ((0, P0), (1, P1), (2, P2), (3, P3)):
            M = _unfold(P, mode)
            G = (M @ M.T).astype(np.float32)
            newU.append(_top_evecs(G, RANKS[mode]))
        U0, U1, U2, U3 = newU

    logits = core.reshape(-1) @ fc_w.T + fc_b
    e = np.exp(logits - logits.max())
    return (e / e.sum()).astype(np.float32)
